# revision 25
# baseline (speedup 1.0000x reference)
"""Trainium2 Bass kernel for BinaryReflectanceGate (gnn_message_passing).

Math (reference):
    feat = [pos, refl]                    # [N,4]
    h1 = relu(feat @ W1 + b1)             # [N,16]
    h2 = relu(h1 @ W2 + b2)               # [N,16]
    smax = segment_max(h2, batch, B=64)   # [64,16]
    logits = smax @ Wg + bg               # [64,2]
    gate = softmax((logits + gumbels)/tau)[:, 1]
    out = gate[batch] * refl              # [N]

Kernel strategy (8 cores, data parallel over whole samples):
  - batch is sorted; core k owns segments [8k, 8k+8).  Each segment is
    padded to a uniform S_seg (multiple of 1024; pad points replicate the
    segment's first point so the max is unchanged).  Segment boundaries
    land on 512-point (group) boundaries, so inside a 4096-pt tile a
    boundary only splits the partition dim.
  - 8-group interleaved layout: 8 groups of 512 points share the 128
    partitions (partition = 32a+4g+f for layer-1 rhs, 16g+ch for hidden),
    with block-diagonal weights -> full contraction per matmul.
  - bf16 matmuls (1 cycle/row); feat ships as bf16 (8 B/point).
  - The inter-layer relu+bias pass and the segment-max pass are the
    bottleneck: every z element is born fp32 in PSUM, the Pool engine
    cannot access PSUM, and ACT/DVE process 1 elem/lane/cycle.  To widen
    the bottleneck, a slice of z2 chunks is evacuated PSUM->SBUF by the
    (otherwise idle) DMA engines; Pool folds those into a per-segment
    running max that DVE reduces once per segment.  The rest of the max
    runs directly on DVE; relu runs mostly on ACT with a small DVE share.
  - Boundary tiles are reduced whole ([128,512]) into "split" mini-slots
    whose partition ranges belong to two segments; the per-segment fold
    fixes them up with partition-sliced tensor_max ops.
  - softmax over 2 classes == sigmoid of the logit difference; relu/b2
    are deferred past the segment max (both monotone).
  - refl and out live as bf16 [128, C]; the per-segment gate scale runs
    on Pool; out is converted to fp32 on the host.
"""
import sys
sys.path.insert(0, "/opt/trn_rl_repo")

import numpy as np
import ml_dtypes
import concourse.bass as bass
import concourse.bacc as bacc
from concourse import mybir
from concourse.tile import TileContext
from concourse.bass_utils import run_bass_kernel_spmd

N = 4_194_304
B = 64
H = 16
NCORES = 8
SEGS_PER_CORE = B // NCORES  # 8
TILE_PTS = 4096              # one [128,512] z tile
CHUNK_PTS = 8192             # one [128,1024] z tile pair

F32 = mybir.dt.float32
F32R = mybir.dt.float32r
BF16 = mybir.dt.bfloat16
BF = ml_dtypes.bfloat16
MAX = mybir.AluOpType.max
ADD = mybir.AluOpType.add

# relu on DVE for chunks with k % RELU_DVE_EVERY == RELU_DVE_EVERY-1
# (DVE is the binding engine here, so keep all relu on ACT)
RELU_DVE_EVERY = 1 << 30
# DMA cannot read PSUM on this stack, so the Pool max chain is disabled
EVAC_QUOTA = 0


def _tile_info(t, S_seg):
    """(seg_lo, seg_hi, m) for tile t: if seg_lo==seg_hi the tile is fully in
    seg_lo; else partitions [0,16m) belong to seg_lo, [16m,128) to seg_hi."""
    p0, p1 = t * TILE_PTS, (t + 1) * TILE_PTS
    s0 = p0 // S_seg
    s1 = (p1 - 1) // S_seg
    if s0 == s1:
        return s0, s0, None
    m = (S_seg * (s0 + 1) - p0) // 512
    return s0, s1, m


def _build_plan(S_seg):
    """Static per-core schedule of segment-max ops.

    ops[k] = ordered list of
      ("dve_chunk", slot)            whole [128,1024] DVE reduce
      ("evac", s, first)             DMA z2 -> SBUF; Pool folds into R_s
      ("rflush", s, slot)            DVE reduce of R_s into a slot
      ("dve_tile", h, slot)          [128,512] DVE reduce of tile half h
    seg_main[s] = (lo, hi) full-partition slot range for the fold
    seg_fix[s]  = [(slot, mask_col)] partition-masked extra slots; engines
        cannot address arbitrary partition windows, so the fold applies
        these via full-width  max(red, minis[:,slot] + mask)  with a
        host-built 0/-1e30 mask column
    masks       = list of (p0, p1) valid-partition ranges per mask column
    seg_last[s] = chunk index after which segment s is complete
    """
    NCHUNK = S_seg * SEGS_PER_CORE // CHUNK_PTS
    ops = [[] for _ in range(NCHUNK)]
    seg_slots = [[] for _ in range(SEGS_PER_CORE)]
    seg_fix = [[] for _ in range(SEGS_PER_CORE)]
    seg_last = [0] * SEGS_PER_CORE
    seg_has_r = [False] * SEGS_PER_CORE
    nslot = 0
    cycle = 0
    n_evac = 0
    masks = []

    def mask_col(p0, p1):
        if (p0, p1) not in masks:
            masks.append((p0, p1))
        return masks.index((p0, p1))

    chunk_seg = []
    for k in range(NCHUNK):
        a = _tile_info(2 * k, S_seg)
        b = _tile_info(2 * k + 1, S_seg)
        chunk_seg.append(a[0] if a[0] == a[1] == b[0] == b[1] else None)

    def rflush(s, k):
        nonlocal nslot
        if seg_has_r[s]:
            ops[k].append(("rflush", s, nslot))
            seg_slots[s].append(nslot)
            seg_last[s] = max(seg_last[s], k)
            nslot += 1
            seg_has_r[s] = False

    for k in range(NCHUNK):
        s = chunk_seg[k]
        if s is not None:
            evac = (cycle % 2 == 0) and n_evac < EVAC_QUOTA
            cycle += 1
            if evac:
                n_evac += 1
                ops[k].append(("evac", s, not seg_has_r[s]))
                seg_has_r[s] = True
                seg_last[s] = max(seg_last[s], k)
            else:
                ops[k].append(("dve_chunk", nslot))
                seg_slots[s].append(nslot)
                seg_last[s] = max(seg_last[s], k)
                nslot += 1
            continue
        for h in (0, 1):
            s0, s1, m = _tile_info(2 * k + h, S_seg)
            rflush(s0, k)          # R-slot precedes the split slot
            ops[k].append(("dve_tile", h, nslot))
            if m is None:
                seg_slots[s0].append(nslot)
            else:
                seg_fix[s0].append((nslot, mask_col(0, 16 * m)))
                seg_fix[s1].append((nslot, mask_col(16 * m, 128)))
                seg_last[s1] = max(seg_last[s1], k)
            seg_last[s0] = max(seg_last[s0], k)
            nslot += 1
    rflush(SEGS_PER_CORE - 1, NCHUNK - 1)

    seg_main = []
    for s in range(SEGS_PER_CORE):
        sl = seg_slots[s]
        assert sl, f"segment {s} has no full slots"
        assert sl == list(range(sl[0], sl[-1] + 1)), f"slots not contiguous {s}"
        seg_main.append((sl[0], sl[-1] + 1))
    return ops, seg_main, seg_fix, seg_last, nslot, masks


def _build_program(S_seg):
    S_core = S_seg * SEGS_PER_CORE
    NCHUNK = S_core // CHUNK_PTS
    NQ = (NCHUNK + 1) // 2       # 16384-pt feature blocks (last may be half)
    C = S_core // 128            # refl/out cols
    W = S_seg // 128             # cols per segment
    ops, seg_main, seg_fix, seg_last, nslot, masks = _build_plan(S_seg)
    NMASK = max(len(masks), 1)

    nc = bacc.Bacc()
    bm_d = nc.declare_dram_parameter("bmask", [128, NMASK], F32,
                                     isOutput=False)

    feat_d = nc.declare_dram_parameter("feat", [NQ, 128, 512], BF16,
                                       isOutput=False)
    refl_d = nc.declare_dram_parameter("refl", [128, C], BF16, isOutput=False)
    gdb_d = nc.declare_dram_parameter("gdb", [1, 8], F32, isOutput=False)
    # w1q | w2b | w1x concatenated: one DMA on the startup critical path
    wc_d = nc.declare_dram_parameter("wcat", [128, 384], BF16, isOutput=False)
    b1_d = nc.declare_dram_parameter("b1r", [128, 1], F32, isOutput=False)
    b2_d = nc.declare_dram_parameter("b2r", [1, 16], F32, isOutput=False)
    wd_d = nc.declare_dram_parameter("wdr", [1, 16], F32, isOutput=False)
    id_d = nc.declare_dram_parameter("ident", [128, 128], F32, isOutput=False)
    out_d = nc.declare_dram_parameter("out", [128, C], BF16, isOutput=True)

    with TileContext(nc) as tc:
        with tc.tile_pool(name="consts", bufs=1) as consts, \
             tc.tile_pool(name="big", bufs=1) as big, \
             tc.tile_pool(name="feat", bufs=3) as featp, \
             tc.tile_pool(name="h1", bufs=3) as h1p, \
             tc.tile_pool(name="ev", bufs=3) as epool, \
             tc.tile_pool(name="rp", bufs=2) as rpool, \
             tc.tile_pool(name="fin", bufs=1) as fin, \
             tc.tile_pool(name="z1", bufs=2, space="PSUM") as z1p, \
             tc.tile_pool(name="z2", bufs=2, space="PSUM") as z2p:

            wcat = consts.tile([128, 384], BF16)
            b1t = consts.tile([128, 1], F32)
            b2r = consts.tile([1, 16], F32)
            wdr = consts.tile([1, 16], F32)
            gdbt = consts.tile([1, 8], F32)
            ident = consts.tile([128, 128], F32)
            maskt = consts.tile([128, NMASK], F32)

            reflt = big.tile([128, C], BF16)
            outt = big.tile([128, C], BF16)
            minis = fin.tile([128, nslot], F32)

            ones1 = consts.tile([1, 128], F32)
            nc.vector.memset(ones1, 1.0)

            # preload the ACT table set (sigmoid set also contains relu)
            preact = fin.tile([1, 1], F32)
            nc.vector.memset(preact, 0.0)
            nc.scalar.activation(out=preact, in_=preact[:],
                                 func=mybir.ActivationFunctionType.Sigmoid,
                                 bias=0.0, scale=1.0)
            nc.scalar.activation(out=preact, in_=preact[:],
                                 func=mybir.ActivationFunctionType.Relu,
                                 bias=0.0, scale=1.0)

            seg_rows = {}
            seg_R = {}

            seg_red = {}

            def finish_segment_stage1a(s):
                # fold the mini slots (DVE); emitted right at the segment's
                # last contributing chunk
                lo, hi = seg_main[s]
                red = fin.tile([128, 1], F32, tag=f"red{s}")
                nc.vector.reduce_max(red, minis[:, lo:hi],
                                     axis=mybir.AxisListType.X)
                for (slot, col) in seg_fix[s]:
                    fx = fin.tile([128, 1], F32, tag="fx")
                    nc.vector.tensor_add(fx, minis[:, slot:slot + 1],
                                         maskt[:, col:col + 1])
                    nc.vector.tensor_max(red, red, fx)
                seg_red[s] = red

            def finish_segment_stage1b(s):
                # group-combine via PE transpose; deferred one chunk so the
                # in-order PE stream never waits on the DVE fold
                red = seg_red[s]
                tp = z2p.tile([1, 128], F32, tag="z2c")
                nc.tensor.transpose(tp, red[:], ident[:])
                row16 = fin.tile([1, 16], F32, tag=f"row{s}")
                nc.vector.reduce_max(
                    row16, tp.rearrange("one (g ch) -> one ch g", g=8),
                    axis=mybir.AxisListType.X)
                seg_rows[s] = row16

            def finish_segment_stage2(s):
                # the last segment's chain is tail-exposed: keep it on
                # DVE/ACT/PE (no Pool q7 launches, fewer cross-engine hops)
                tail = s == SEGS_PER_CORE - 1
                eng = nc.vector if tail else nc.gpsimd
                row16 = seg_rows[s]
                srel = fin.tile([1, 16], F32, tag=f"srel{s}")
                eng.tensor_add(srel, row16, b2r[:])
                eng.tensor_scalar_max(srel, srel, 0.0)
                eng.tensor_mul(srel, srel, wdr[:])
                logit = fin.tile([1, 1], F32, tag=f"lg{s}")
                nc.vector.reduce_sum(logit, srel, axis=mybir.AxisListType.X)
                gate1 = fin.tile([1, 1], F32, tag=f"g{s}")
                nc.scalar.activation(out=gate1, in_=logit[:],
                                     func=mybir.ActivationFunctionType.Sigmoid,
                                     bias=gdbt[0:1, s:s + 1], scale=1.0)
                if tail:
                    # broadcast gate across partitions with a K=1 matmul
                    gbcp = z2p.tile([128, 1], F32, tag="z2c")
                    nc.tensor.matmul(gbcp[:], lhsT=ones1[:], rhs=gate1[:],
                                     start=True, stop=True)
                    nc.vector.tensor_scalar_mul(
                        outt[:, W * s:W * (s + 1)],
                        reflt[:, W * s:W * (s + 1)],
                        gbcp[:, 0:1])
                else:
                    gbc = fin.tile([128, 1], F32, tag=f"gb{s}")
                    nc.gpsimd.partition_broadcast(gbc, gate1[:])
                    nc.gpsimd.tensor_scalar_mul(
                        outt[:, W * s:W * (s + 1)],
                        reflt[:, W * s:W * (s + 1)],
                        gbc[:, 0:1])
                nc.sync.dma_start(out=out_d[:, W * s:W * (s + 1)],
                                  in_=outt[:, W * s:W * (s + 1)])

            refl_dma_done = set()
            z1_tiles = {}
            fq_ref = [None]

            def emit_L1(k):
                # layer-1 matmuls for chunk k, emitted one chunk ahead of
                # the rest of chunk k's pipeline so the in-order PE stream
                # never stalls waiting on the relu engines
                if k % 2 == 0:
                    fqt = featp.tile([128, 512], BF16, tag="fq")
                    nc.sync.dma_start(out=fqt, in_=feat_d[k // 2])
                    fq_ref[0] = fqt
                if k == 0:
                    # minimal critical-path set for chunk 0
                    nc.sync.dma_start(out=wcat, in_=wc_d[:])
                    nc.sync.dma_start(out=b1t, in_=b1_d[:])
                if k == 2:
                    nc.sync.dma_start(out=ident, in_=id_d[:])
                    nc.sync.dma_start(out=b2r, in_=b2_d[:])
                    nc.sync.dma_start(out=wdr, in_=wd_d[:])
                    nc.sync.dma_start(out=gdbt, in_=gdb_d[:])
                    nc.sync.dma_start(out=maskt, in_=bm_d[:])
                fq = fq_ref[0]
                z1c = z1p.tile([128, 1024], F32, tag="z1c")
                z1_tiles[k] = z1c
                for j in range(2):
                    a = 2 * (k % 2) + j
                    if a < 3:
                        nc.tensor.matmul(
                            z1c[:, 512 * j:512 * (j + 1)],
                            lhsT=wcat[32 * a:32 * (a + 1), 0:128],
                            rhs=fq[32 * a:32 * (a + 1), :],
                            start=True, stop=True)
                    else:
                        # base partition 96 is illegal; contract K=64 from
                        # base 64 with a zero top half in the weights
                        nc.tensor.matmul(
                            z1c[:, 512 * j:512 * (j + 1)],
                            lhsT=wcat[64:128, 256:384],
                            rhs=fq[64:128, :],
                            start=True, stop=True)

            emit_L1(0)
            for k in range(NCHUNK):
                if k + 1 < NCHUNK:
                    emit_L1(k + 1)
                s_here = (k * CHUNK_PTS) // S_seg
                if s_here not in refl_dma_done:
                    refl_dma_done.add(s_here)
                    nc.sync.dma_start(
                        out=reflt[:, W * s_here:W * (s_here + 1)],
                        in_=refl_d[:, W * s_here:W * (s_here + 1)])

                z1c = z1_tiles.pop(k)
                h1c = h1p.tile([128, 1024], BF16, tag="h1c")
                if k % RELU_DVE_EVERY == RELU_DVE_EVERY - 1:
                    nc.vector.tensor_scalar(
                        out=h1c, in0=z1c[:], scalar1=b1t[:, 0:1],
                        scalar2=0.0, op0=ADD, op1=MAX)
                else:
                    nc.scalar.activation(
                        out=h1c, in_=z1c[:],
                        func=mybir.ActivationFunctionType.Relu,
                        bias=b1t[:, 0:1], scale=1.0)
                z2c = z2p.tile([128, 1024], F32, tag="z2c")
                for j in range(2):
                    nc.tensor.matmul(z2c[:, 512 * j:512 * (j + 1)],
                                     lhsT=wcat[:, 128:256],
                                     rhs=h1c[:, 512 * j:512 * (j + 1)],
                                     start=True, stop=True)

                for op in ops[k]:
                    if op[0] == "dve_chunk":
                        nc.vector.reduce_max(minis[:, op[1]:op[1] + 1],
                                             z2c[:], axis=mybir.AxisListType.X)
                    elif op[0] == "evac":
                        _, s, first = op
                        ev = epool.tile([128, 1024], F32, tag="E")
                        nc.sync.dma_start(out=ev, in_=z2c[:])
                        if first:
                            R = rpool.tile([128, 1024], F32, tag="R")
                            seg_R[s] = R
                            nc.gpsimd.tensor_copy(R[:], ev[:])
                        else:
                            R = seg_R[s]
                            nc.gpsimd.tensor_max(R[:], ev[:], R[:])
                    elif op[0] == "rflush":
                        _, s, slot = op
                        nc.vector.reduce_max(minis[:, slot:slot + 1],
                                             seg_R[s][:],
                                             axis=mybir.AxisListType.X)
                    else:
                        _, h, slot = op
                        nc.vector.reduce_max(
                            minis[:, slot:slot + 1],
                            z2c[:, 512 * h:512 * (h + 1)],
                            axis=mybir.AxisListType.X)

                for s in range(SEGS_PER_CORE):
                    if seg_last[s] == k:
                        finish_segment_stage1a(s)
                    if seg_last[s] == k - 1:
                        finish_segment_stage1b(s)
                    if seg_last[s] == k - 2:
                        finish_segment_stage2(s)
            for s in range(SEGS_PER_CORE):
                if seg_last[s] == NCHUNK - 1:
                    finish_segment_stage1b(s)
            for s in range(SEGS_PER_CORE):
                if seg_last[s] >= NCHUNK - 2:
                    finish_segment_stage2(s)

    nc.compile()
    return nc


_CACHE = {}


def _program(S_seg):
    if S_seg not in _CACHE:
        _CACHE[S_seg] = _build_program(S_seg)
    return _CACHE[S_seg]


def _prep_inputs(pos, reflectance, batch, gumbels, W1, b1, W2, b2, Wg, bg):
    pos = np.asarray(pos, np.float32)
    reflectance = np.asarray(reflectance, np.float32)
    batch = np.asarray(batch, np.int32)
    gumbels = np.asarray(gumbels, np.float32)
    W1, b1 = np.asarray(W1, np.float32), np.asarray(b1, np.float32)
    W2, b2 = np.asarray(W2, np.float32), np.asarray(b2, np.float32)
    Wg, bg = np.asarray(Wg, np.float32), np.asarray(bg, np.float32)

    bounds = np.searchsorted(batch, np.arange(B + 1), side="left")
    seg_len = np.diff(bounds)
    S_seg = int(-(-max(1, seg_len.max()) // 1024) * 1024)
    S_core = S_seg * SEGS_PER_CORE
    NCHUNK = S_core // CHUNK_PTS
    NQ = (NCHUNK + 1) // 2
    C = S_core // 128

    feat = np.concatenate([pos, reflectance[:, None]], axis=1)  # [N,4]

    w1q = np.zeros((128, 128), np.float32)
    w2b = np.zeros((128, 128), np.float32)
    for g in range(8):
        w2b[16 * g:16 * (g + 1), 16 * g:16 * (g + 1)] = W2
    for a in range(4):
        for g in range(8):
            w1q[32 * a + 4 * g:32 * a + 4 * (g + 1), 16 * g:16 * (g + 1)] = W1
    w1x = np.zeros((128, 128), np.float32)
    w1x[96:128] = w1q[96:128]
    b1r = np.tile(b1, 8)[:, None].astype(np.float32)
    b2r = b2[None, :].astype(np.float32)
    wdr = (Wg[:, 1] - Wg[:, 0])[None, :].astype(np.float32)
    ident = np.eye(128, dtype=np.float32)
    gdel = (bg[1] - bg[0]) + gumbels[:, 1] - gumbels[:, 0]  # [B]

    masks = _build_plan(S_seg)[5]
    bmask = np.full((128, max(len(masks), 1)), -1e30, np.float32)
    for col, (p0, p1) in enumerate(masks):
        bmask[p0:p1, col] = 0.0

    in_maps = []
    for core in range(NCORES):
        fpad = np.zeros((SEGS_PER_CORE, S_seg, 4), np.float32)
        rpad = np.zeros((SEGS_PER_CORE, S_seg), np.float32)
        for s in range(SEGS_PER_CORE):
            seg = SEGS_PER_CORE * core + s
            lo, hi = bounds[seg], bounds[seg + 1]
            n = hi - lo
            if n > 0:
                fpad[s, :n] = feat[lo:hi]
                fpad[s, n:] = feat[lo]        # replicate first point
                rpad[s, :n] = reflectance[lo:hi]
        # feat_host[q, 32a + 4g+f, c] = fpad[point (4q+a)*4096 + g*512 + c, f]
        fflat = np.zeros((NQ * 16384, 4), np.float32)
        fflat[:S_core] = fpad.reshape(S_core, 4)
        fh = (fflat.reshape(NQ, 4, 8, 512, 4)      # q, a, g, c, f
                   .transpose(0, 1, 2, 4, 3)       # q, a, g, f, c
                   .reshape(NQ, 128, 512)).astype(BF)
        rh = np.ascontiguousarray(
            rpad.reshape(C, 128).T).astype(BF)
        gdb = gdel[SEGS_PER_CORE * core:SEGS_PER_CORE * (core + 1)][None, :]
        in_maps.append({
            "feat": np.ascontiguousarray(fh),
            "refl": rh,
            "gdb": np.ascontiguousarray(gdb.astype(np.float32)),
            "wcat": np.ascontiguousarray(np.concatenate(
                [w1q, w2b, w1x], axis=1).astype(BF)), "b1r": b1r,
            "b2r": b2r, "wdr": wdr, "ident": ident, "bmask": bmask,
        })
    return in_maps, bounds, S_seg


_LAST_S_CAP = None


def _run(trace=False, **inputs):
    global _LAST_S_CAP
    in_maps, bounds, S_seg = _prep_inputs(**inputs)
    _LAST_S_CAP = S_seg
    nc = _program(S_seg)
    res = run_bass_kernel_spmd(nc, in_maps, list(range(NCORES)), trace=trace)
    out = np.empty(N, np.float32)
    S_core = S_seg * SEGS_PER_CORE
    for core in range(NCORES):
        o = res.results[core]["out"]              # [128, C] bf16
        flat = np.asarray(o).T.reshape(S_core).astype(np.float32)
        for s in range(SEGS_PER_CORE):
            seg = SEGS_PER_CORE * core + s
            lo, hi = bounds[seg], bounds[seg + 1]
            if hi > lo:
                out[lo:hi] = flat[s * S_seg:s * S_seg + hi - lo]
    return out, res


def kernel(**inputs) -> np.ndarray:
    out, _ = _run(trace=False, **inputs)
    return out
